# revision 39
# baseline (speedup 1.0000x reference)
"""Trainium2 Bass kernel for the combined loss (KL + CE + InfoNCE + focal + adv CE).

v3 strategy (8 NeuronCores, data-parallel over the batch):
  - InfoNCE exploits Gram symmetry: block-rows 0..3 compute circulant
    distances 0..32, block-rows 32..35 only 0..31 (their d=32 values come
    from the mirror block's colsum); all remaining mirrored distances come
    from PE ones-matmul column sums of the exp tiles.
  - Gram matmuls run in fp8(e4m3) DoubleRow mode: K=256 contraction in one
    pass at 2 cols/cycle.  Features are normalized, pre-scaled by
    sqrt(log2e/(32*T)) and packed [128, 2, EXT] on the host, so the PSUM
    values are y with exp(G/T) = 2^(32y).
  - Gram exp runs on ACT (scale=1/(s2*T)) with row-sum accumulators.
  - CE/KL/adv exp row-sums are split between ACT and a custom 2-instruction
    DVE exp pipeline (quartic poly in 2^y then 5 squarings, scale folded
    into the coefficients), balancing the two engines.
  - o/m/a ship as bf16; targets are gathered on the host (GO/GA stay host-side).
  - Colsum PSUM rows copy out through DVE; per-sample epilogue on host.
  - Positive-pair logits are read from the SBUF exp tiles (log on host) so no
    late DVE read ever pins a PSUM gram buffer; CE/KL units and colsum
    regions are spread across chunks to keep all engines fed.
"""

import numpy as np
import ml_dtypes
from operator import add as _add

import concourse.bacc as bacc
import concourse.tile as tile
from concourse import mybir
from concourse.bass_utils import run_bass_kernel_spmd

import concourse.dve_ops as DO
from concourse.dve_spec import (Spec, Src0, C0 as _C0, C1 as _C1, C2 as _C2,
                                C3 as _C3, Zero, One, lower as _dve_lower,
                                sq as _sq, _spill_c3_to_src1, _has_src1)
from concourse.dve_uop import DveOpSpec

F32 = mybir.dt.float32
BF16 = mybir.dt.bfloat16
FP8 = mybir.dt.float8e4
AF = mybir.ActivationFunctionType
ALU = mybir.AluOpType
DR = mybir.MatmulPerfMode.DoubleRow

NCORES = 8
B, C, D = 4096, 1000, 256
RB = B // NCORES          # 512 rows of the [B, C] tensors per core
NT = RB // 128            # 4 row-tiles per core
N2 = 2 * B                # 8192 infoNCE rows
NDIST = 33                # circulant distances d = 0..32 per block-row
SPAN = NDIST * 128        # 4224 columns per block-row sweep
L_ROWS = [0, 1, 2, 3, 32, 33, 34, 35]   # local block-row indices (all cores)
# d=32 blocks are computed only for l<4; the mirror rows (l>=32) get them
# via the colsum path, so those sweeps stop at distance 31.
EXT = 35 * 128 + 4096     # 8576 extended local columns
CHW = 1536                # gram/exp chunk width (3 PSUM banks)
NCHK = (EXT + CHW - 1) // CHW           # 6 chunks
RGW = 512                 # colsum accumulation region width (1 PSUM bank)
NREG = (EXT + RGW - 1) // RGW           # 17 regions (last one 384 wide)

KL_TEMP = 4.0
KL_INTERP = 0.5
NCE_TEMP = 0.07
LOG2E = float(np.log2(np.e))
S2 = LOG2E / (32.0 * NCE_TEMP)      # feature pre-scale^2; y = S2 * cos_sim
ACT_SCALE = 1.0 / (S2 * NCE_TEMP)   # exp(ACT_SCALE * y) = exp(G/T)
NEG_DIAG = -1.875                   # diag mask add (y_diag ~ -1.231)

# exp2 quartic: 2^y ~ c0*(1 + y(b1 + y(b2 + y(b3 + y*b4)))) on [-1.30, 0.67]
PB1, PB2, PB3, PB4 = 0.69336677, 0.24124203, 0.05543758, 0.00758271
PC0_32 = 0.9979927195289331         # c0^32

OMA_W = 6016              # o|m|a bf16 (2000 each) + 16 pad


def _register_dve(name, spec):
    if name in DO._SUB_OPCODE_FOR_NAME:
        return next(o for o in DO.OPS if o.name == name)
    op = DO.DveOp(name, spec, subdim=False, uops_sha={})
    DO.OPS.append(op)
    DO._SUB_OPCODE_FOR_NAME[name] = DO._CUSTOM_DVE_ROW_BASE + len(DO.OPS) - 1
    DO.CUSTOM_DVE_SPECS[name] = spec
    for ver in ("v3", "v4"):
        s = DveOpSpec(name=name, opcode=DO.get_dve_sub_opcode(name),
                      uops=_dve_lower(spec, ver=ver), rd1_en=_has_src1(spec))
        op.uops_sha[ver] = s.sha(ver)
    return op


def _ref_exp2pa(in0, in1, s0, s1, imm2):
    x = in0.astype(np.float32)
    return (1.0 + x * (s0 + x * (s1 + x * (imm2 + x * in1)))).astype(np.float32)


def _ref_exp2pb(in0, in1, s0, s1, imm2):
    b = ((in0.astype(np.float64) ** 32) * s0).astype(np.float32)
    return b, b.reshape(b.shape[0], -1).sum(axis=-1, keepdims=True).astype(
        np.float32)


EXP2PA = _register_dve("EXP2PA_ANT", Spec(
    body=_spill_c3_to_src1(
        One + Src0 * (_C0 + Src0 * (_C1 + Src0 * (_C2 + Src0 * _C3)))),
    reference=_ref_exp2pa))
EXP2PB = _register_dve("EXP2PB_ANT", Spec(
    body=_sq(_sq(_sq(_sq(_sq(Src0))))) * _C0, accum=_add, accum_init=Zero,
    reference=_ref_exp2pb))


def _poly_coefs(scale):
    """Fold an input pre-scale into the exp2 quartic: R(s*x) coefficients."""
    s = float(scale)
    return PB1 * s, PB2 * s * s, PB3 * s ** 3, PB4 * s ** 4


def _pair_table():
    """(l, c, a, b) for every (block-row, chunk) intersection, chunk-major."""
    pairs = []
    for c in range(NCHK):
        c0, c1 = CHW * c, min(CHW * (c + 1), EXT)
        for l in L_ROWS:
            s = 128 * l
            e = 128 * l + (SPAN if l < 4 else 4096)
            a, b = max(c0, s), min(c1, e)
            if b > a:
                pairs.append((l, c, a, b))
    return pairs


PAIRS = _pair_table()
NSLOT = len(PAIRS)        # 28


def _region_pieces():
    """region -> list of (pair_idx, p0, p1) colsum pieces; a full-covering
    piece (if any) is moved to the front so no zero-init matmul is needed."""
    reg = {}
    for idx, (l, c, a, b) in enumerate(PAIRS):
        a2 = max(a, 128 * l + 128)        # exclude d=0 (diagonal block)
        # l<4: include d=32 in the colsum (it feeds the l+32 mirror rows);
        # l>=32: sweep already stops at d=31.
        b2 = min(b, 128 * l + 4096 + (128 if l < 4 else 0))
        if b2 <= a2:
            continue
        r0, r1 = a2 // RGW, (b2 - 1) // RGW
        for r in range(r0, r1 + 1):
            p0, p1 = max(a2, RGW * r), min(b2, RGW * (r + 1))
            reg.setdefault(r, []).append((idx, p0, p1))
    for r, pieces in reg.items():
        end = min(RGW * (r + 1), EXT)
        full = [k for k, (_, p0, p1) in enumerate(pieces)
                if p0 == RGW * r and p1 == end]
        if full:
            k = full[0]
            pieces.insert(0, pieces.pop(k))
    return reg


REGION_PIECES = _region_pieces()

# CE/KL exp units: (tile, kind); kind 0=exp(o), 1=exp(o/4), 2=exp(m/4)->em,
# 3=exp(a).  Stat slot column = 33 + kind*4 + tile (S1|ST|SM|SA), PP at 49+t.
CEKL_ON_ACT = {(0, 0), (0, 2), (1, 2), (2, 2), (3, 2)}   # 5 units on ACT
OUT_W = 53


def _build_module():
    nc = bacc.Bacc("TRN2", target_bir_lowering=False, debug=False)

    oma_d = nc.dram_tensor("oma", [RB, OMA_W], mybir.dt.uint8,
                           kind="ExternalInput")
    hp_d = nc.dram_tensor("hp", [128, 2 * EXT], mybir.dt.uint8,
                          kind="ExternalInput")
    out_d = nc.dram_tensor("out", [128, OUT_W], F32, kind="ExternalOutput")
    cs_d = nc.dram_tensor("cs", [1, EXT], F32, kind="ExternalOutput")

    # packed constants: ident f32 | identb bf16 | negidb bf16 | onesb bf16 |
    # zerob bf16 | poly consts f32 (b4 variants)
    cpack = np.concatenate([
        np.eye(128, dtype=np.float32).view(np.uint8).reshape(128, -1),
        np.eye(128).astype(ml_dtypes.bfloat16).view(np.uint8).reshape(128, -1),
        (NEG_DIAG * np.eye(128)).astype(ml_dtypes.bfloat16).view(
            np.uint8).reshape(128, -1),
        np.ones((128, 128)).astype(ml_dtypes.bfloat16).view(
            np.uint8).reshape(128, -1),
        np.zeros((128, RGW)).astype(ml_dtypes.bfloat16).view(
            np.uint8).reshape(128, -1),
        np.tile(np.array([[_poly_coefs(1.0)[3],
                           _poly_coefs(LOG2E / 32.0)[3],
                           _poly_coefs(LOG2E / 128.0)[3]]], np.float32),
                (128, 1)).view(np.uint8).reshape(128, -1),
    ], axis=1)
    cpack_d = nc.inline_tensor(cpack, "cpack_c")

    with tile.TileContext(nc) as tc:
        with (
            tc.tile_pool(name="persist", bufs=1) as persist,
            tc.tile_pool(name="io", bufs=1) as iop,
            tc.tile_pool(name="em", bufs=4) as emp,
            tc.tile_pool(name="scr", bufs=2) as scrp,
            tc.tile_pool(name="qt", bufs=2) as qtp,
            tc.tile_pool(name="et", bufs=16) as etp,
            tc.tile_pool(name="vec", bufs=1) as vecp,
            tc.tile_pool(name="gp", bufs=2, space="PSUM") as gpp,
            tc.tile_pool(name="cs", bufs=2, space="PSUM") as csp,
        ):
            dma = nc.sync.dma_start

            cpack_t = persist.tile([128, cpack.shape[1]], mybir.dt.uint8,
                                   tag="cpack")
            ident_t = cpack_t[:, 0:512].bitcast(F32)
            identb_t = cpack_t[:, 512:768].bitcast(BF16)
            negidb_t = cpack_t[:, 768:1024].bitcast(BF16)
            onesb_t = cpack_t[:, 1024:1280].bitcast(BF16)
            zerob_t = cpack_t[:, 1280:1280 + 2 * RGW].bitcast(BF16)
            b4c_t = cpack_t[:, 1280 + 2 * RGW:1280 + 2 * RGW + 12].bitcast(F32)

            hp8 = persist.tile([128, 2, EXT], FP8, tag="hp8")
            hp8u = hp8.bitcast(mybir.dt.uint8)

            oma_ts, o_ts, m_ts, a_ts = [], [], [], []
            for t in range(NT):
                oma_t = iop.tile([128, OMA_W], mybir.dt.uint8, tag=f"oma{t}")
                oma_ts.append(oma_t)
                o_ts.append(oma_t[:, 0:2000].bitcast(BF16))
                m_ts.append(oma_t[:, 2000:4000].bitcast(BF16))
                a_ts.append(oma_t[:, 4000:6000].bitcast(BF16))

            # progressive feature pieces so the first gram starts early
            P0, P1, P2 = 512, 1536, 4608
            dma(out=cpack_t[:], in_=cpack_d[:])

            def dma_hp(lo, hi):
                for j in (0, 1):
                    dma(out=hp8u[:, j:j + 1, lo:hi],
                        in_=hp_d[:, j * EXT + lo:j * EXT + hi])

            def dma_oma(t, lo, hi):
                # split across queues so one tile isn't serialized on a ring
                rsl = slice(t * 128, (t + 1) * 128)
                step = (hi - lo + 1) // 2
                for x in range(lo, hi, step):
                    e = min(x + step, hi)
                    dma(out=oma_ts[t][:, x:e], in_=oma_d[rsl, x:e])

            dma_oma(0, 0, 2000)     # o of tile 0: first ACT work
            dma_hp(0, P0)
            dma_oma(0, 2000, 4000)
            dma_hp(P0, P1)
            dma_oma(0, 4000, 6016)
            dma_oma(1, 0, 4000)
            dma_hp(P1, P2)
            dma_oma(1, 4000, 6016)
            dma_oma(2, 0, 4000)
            dma_oma(3, 0, 4000)
            dma_hp(P2, EXT)
            dma_oma(2, 4000, 6016)
            dma_oma(3, 4000, 6016)

            out_sb = vecp.tile([128, OUT_W], F32, tag="out_sb")
            rs_sl = out_sb[:, 0:NSLOT]
            rs_x = out_sb[:, 28:29]          # second slot of the split pair 0
            pos_sb = out_sb[:, 29:33]
            st_sb = out_sb[:, 33:OUT_W]
            cs_sb = vecp.tile([1, EXT], F32, tag="cs_sb")
            pscr = vecp.tile([128, 128], F32, tag="pscr")
            dummy_a = vecp.tile([128, 1000], BF16, tag="dummy_a")  # ACT-only
            dummy_v = vecp.tile([128, 1000], BF16, tag="dummy_v")  # DVE-only

            et_tiles = {}
            em_ts = {}

            def emit_gram(idx):
                l, c, a, b = PAIRS[idx]
                w = b - a
                s_l = 128 * l
                gp = gpp.tile([128, CHW], F32, tag="gp")
                et_tiles[idx] = (gp, None)
                lhsT = hp8[:, :, s_l:s_l + 128]
                for sub in range(0, w, 512):
                    n = min(512, w - sub)
                    d0 = a + sub <= s_l < a + sub + n
                    nc.tensor.matmul(gp[:, sub:sub + n], lhsT,
                                     hp8[:, :, a + sub:a + sub + n],
                                     perf_mode=DR, start=True, stop=not d0,
                                     skip_group_check=True)
                    if d0:
                        off = s_l - a
                        nc.tensor.matmul(gp[:, off:off + 128], negidb_t[:],
                                         identb_t[:], start=False, stop=True,
                                         skip_group_check=True)
                return gp

            def emit_pos(idx):
                # diag of the d=32 block, read from the SBUF exp tile (NOT
                # the PSUM gram — a late DVE read there pins a PSUM buf and
                # stalls the PE/ACT pipeline); host takes log() to recover
                # the positive-pair logit.
                l, c, a, b = PAIRS[idx]
                p0 = 128 * l + 4096
                if l < 4 and a <= p0 < b:
                    off = p0 - a
                    e_t = et_tiles[idx][1]
                    nc.vector.scalar_tensor_tensor(
                        out=pscr[:], in0=e_t[:, off:off + 128], scalar=1.0,
                        in1=ident_t[:], op0=ALU.mult, op1=ALU.mult,
                        accum_out=pos_sb[:, l:l + 1])

            def emit_exp_act(idx, gp, split=False):
                l, c, a, b = PAIRS[idx]
                w = b - a
                e_t = etp.tile([128, CHW], BF16, tag="et")
                et_tiles[idx] = (gp, e_t)
                if split:
                    nc.scalar.activation(e_t[:, 0:512], gp[:, 0:512], AF.Exp,
                                         scale=ACT_SCALE, accum_out=rs_x[:])
                    nc.scalar.activation(e_t[:, 512:w], gp[:, 512:w], AF.Exp,
                                         scale=ACT_SCALE,
                                         accum_out=rs_sl[:, idx:idx + 1])
                else:
                    nc.scalar.activation(e_t[:, :w], gp[:, :w], AF.Exp,
                                         scale=ACT_SCALE,
                                         accum_out=rs_sl[:, idx:idx + 1])

            def emit_cekl_unit(t, kind):
                src = (o_ts[t], o_ts[t], m_ts[t], a_ts[t])[kind]
                slot = st_sb[:, kind * 4 + t:kind * 4 + t + 1]
                on_act = (t, kind) in CEKL_ON_ACT
                if kind == 2:
                    em_t = emp.tile([128, 1000], BF16, tag="em")
                    em_ts[t] = em_t
                    dst = em_t[:]
                else:
                    dst = (dummy_a if on_act else dummy_v)[:, 0:1000]
                if on_act:
                    scale = 1.0 if kind in (0, 3) else 0.25
                    nc.scalar.activation(dst, src[:], AF.Exp, scale=scale,
                                         accum_out=slot)
                else:
                    s = LOG2E / 32.0 if kind in (0, 3) else LOG2E / 128.0
                    b1, b2, b3, _ = _poly_coefs(s)
                    b4col = 1 if kind in (0, 3) else 2
                    q_t = qtp.tile([128, 1000], F32, tag="q")
                    nc.vector._custom_dve(
                        EXP2PA, out=q_t[:], in0=src[:],
                        in1=b4c_t[:, b4col:b4col + 1], s0=b1, s1=b2, imm2=b3)
                    nc.vector._custom_dve(
                        EXP2PB, out=dst, in0=q_t[:], s0=PC0_32,
                        accum_out=slot)

            def emit_cekl_pp(t):
                d_t = scrp.tile([128, 1000], BF16, tag="d")
                nc.vector.tensor_sub(d_t[:], m_ts[t][:], o_ts[t][:])
                nc.vector.scalar_tensor_tensor(
                    out=dummy_v[:, 0:1000], in0=d_t[:], scalar=1.0,
                    in1=em_ts[t][:], op0=ALU.mult, op1=ALU.mult,
                    accum_out=st_sb[:, 16 + t:17 + t])

            def emit_colsum_region(r):
                if r not in REGION_PIECES:
                    return
                pieces = REGION_PIECES[r]
                end = min(RGW * (r + 1), EXT)
                w = end - RGW * r
                ct = csp.tile([128, RGW], F32, tag="cs")
                full0 = (pieces[0][1] == RGW * r and pieces[0][2] == end)
                if not full0:
                    nc.tensor.matmul(ct[:, 0:w], onesb_t[:], zerob_t[:, 0:w],
                                     start=True, stop=False,
                                     skip_group_check=True)
                for k, (idx, p0_, p1_) in enumerate(pieces):
                    _, _, a, _ = PAIRS[idx]
                    e_t = et_tiles[idx][1]
                    nc.tensor.matmul(
                        ct[:, p0_ - RGW * r:p1_ - RGW * r],
                        onesb_t[:], e_t[:, p0_ - a:p1_ - a],
                        start=(k == 0 and full0),
                        stop=(k == len(pieces) - 1),
                        skip_group_check=True)
                nc.vector.tensor_copy(
                    cs_sb[0:1, RGW * r:end], ct[0:1, 0:w])
                dma(out=cs_d[0:1, RGW * r:end],
                    in_=cs_sb[0:1, RGW * r:end])

            # unit emission order: DVE units early (DMA-dependent only) in
            # tile order matching DMA arrival; ACT units fill gram gaps.
            dve_units = [(0, 1), (0, 3), (1, 1), (1, 0), (1, 3), (2, 1),
                         (2, 0), (2, 3), (3, 1), (3, 0), (3, 3)]
            act_units = [(1, 2), (2, 2), (3, 2)]
            pair_of_chunk = [[i for i, p in enumerate(PAIRS) if p[1] == c]
                             for c in range(NCHK)]
            pp_done = 0
            for c in range(NCHK):
                for k, idx in enumerate(pair_of_chunk[c]):
                    gp = emit_gram(idx)
                    # spread the previous chunk's colsum regions across this
                    # chunk's pairs so PE work stays smooth
                    if c >= 1 and k < 3:
                        emit_colsum_region(3 * (c - 1) + k)
                    emit_exp_act(idx, gp, split=(idx == 0))
                    emit_pos(idx)
                    if idx == 0:
                        # gram exp first (earliest data), then tile-0 cekl
                        emit_cekl_unit(0, 0)
                        emit_cekl_unit(0, 2)
                        for _ in range(2):
                            if dve_units:
                                emit_cekl_unit(*dve_units.pop(0))
                    elif k == len(pair_of_chunk[c]) // 2 and c >= 1:
                        if c >= 2 and act_units:
                            emit_cekl_unit(*act_units.pop(0))
                        for _ in range(2):
                            if dve_units:
                                emit_cekl_unit(*dve_units.pop(0))
                if c >= 1:
                    if dve_units:
                        emit_cekl_unit(*dve_units.pop(0))
                    if c >= 2 and pp_done < NT and pp_done in em_ts:
                        emit_cekl_pp(pp_done)
                        pp_done += 1
            for r in range(3 * (NCHK - 1), NREG):
                emit_colsum_region(r)
            for u in act_units:
                emit_cekl_unit(*u)
            for u in dve_units:
                emit_cekl_unit(*u)
            while pp_done < NT:
                if pp_done in em_ts:
                    emit_cekl_pp(pp_done)
                pp_done += 1

            dma(out=out_d[:], in_=out_sb[:])

    nc.compile()
    return nc


_NC = None


def _get_nc():
    global _NC
    if _NC is None:
        _NC = _build_module()
    return _NC


_HOST = {}


def _prep_inputs(output, target, master_net_pred, feat_pooled,
                 feat_pooled_masked, output_adv, target_adv):
    o = np.asarray(output, dtype=np.float32)
    m = np.asarray(master_net_pred, dtype=np.float32)
    a = np.asarray(output_adv, dtype=np.float32)
    tg = np.asarray(target).astype(np.int64)
    ta = np.asarray(target_adv).astype(np.int64)
    f0 = np.asarray(feat_pooled, dtype=np.float32)
    f1 = np.asarray(feat_pooled_masked, dtype=np.float32)
    feats = np.concatenate([f0, f1], axis=0)  # [2B, D]
    feats = feats / np.linalg.norm(feats, axis=1, keepdims=True)
    feats = feats * np.float32(np.sqrt(S2))

    _HOST["GO"] = np.take_along_axis(o, tg[:, None], axis=1)[:, 0]
    _HOST["GA"] = np.take_along_axis(a, ta[:, None], axis=1)[:, 0]

    o_bf = o.astype(ml_dtypes.bfloat16)
    m_bf = m.astype(ml_dtypes.bfloat16)
    a_bf = a.astype(ml_dtypes.bfloat16)

    in_maps = []
    for cc in range(NCORES):
        sl = slice(cc * RB, (cc + 1) * RB)
        rolled = np.roll(feats, -RB * cc, axis=0)
        ext = np.concatenate([rolled, rolled[:EXT - N2]], axis=0)  # [8704, D]
        f8 = np.ascontiguousarray(ext.T).astype(ml_dtypes.float8_e4m3)
        hp = np.concatenate([f8[0:128], f8[128:256]], axis=1)  # [128, 2*EXT]
        oma = np.zeros((RB, OMA_W), dtype=np.uint8)
        oma[:, 0:2000] = np.ascontiguousarray(o_bf[sl]).view(np.uint8)
        oma[:, 2000:4000] = np.ascontiguousarray(m_bf[sl]).view(np.uint8)
        oma[:, 4000:6000] = np.ascontiguousarray(a_bf[sl]).view(np.uint8)
        in_maps.append({"oma": oma, "hp": hp.view(np.uint8)})
    return in_maps


def _combine(results):
    S = np.zeros(N2, dtype=np.float64)
    pos_full = np.zeros(N2, dtype=np.float64)
    arp = np.arange(128)
    for cc, rr in enumerate(results):
        rs = rr["out"][:, 0:NSLOT].astype(np.float64)
        rs[:, 0] += rr["out"][:, 28].astype(np.float64)
        cs = rr["cs"].reshape(-1).astype(np.float64)   # [EXT]
        pos = rr["out"][:, 29:33].astype(np.float64)
        for idx, (l, c, a, b) in enumerate(PAIRS):
            rows = (RB * cc + 128 * l + arp) % N2
            np.add.at(S, rows, rs[:, idx])
        gcols = (np.arange(EXT) + RB * cc) % N2
        np.add.at(S, gcols, cs)
        for l in range(4):
            i = RB * cc + 128 * l + arp
            pos_full[i] = pos[:, l]
            pos_full[i + B] = pos[:, l]
    pos_logit = np.log(pos_full)   # pos slots hold exp(G/T) from the exp tile
    nce_mean = float(np.mean(np.log(S) - pos_logit))

    # CE / KL / focal / adv from per-row stats
    sts = [r["out"][:, 33:OUT_W] for r in results]
    S1 = np.concatenate([st[:, 0:4].T.reshape(-1) for st in sts])
    ST = np.concatenate([st[:, 4:8].T.reshape(-1) for st in sts])
    SM = np.concatenate([st[:, 8:12].T.reshape(-1) for st in sts])
    SA = np.concatenate([st[:, 12:16].T.reshape(-1) for st in sts])
    PP = np.concatenate([st[:, 16:20].T.reshape(-1) for st in sts])
    S1, ST, SM, SA, PP = (x.astype(np.float64)
                          for x in (S1, ST, SM, SA, PP))
    GO = _HOST["GO"].astype(np.float64)
    GA = _HOST["GA"].astype(np.float64)
    ce = np.log(S1) - GO
    adv = np.log(SA) - GA
    kl = PP / (KL_TEMP * SM) - np.log(SM) + np.log(ST)
    pt = np.exp(-ce)
    gamma = np.where(pt < 0.2, 5.0, np.where(pt < 0.5, 3.0, 1.0))
    foc = ((1.0 - pt) ** gamma) * ce
    loss = (KL_INTERP * KL_TEMP * KL_TEMP) * np.mean(kl) / C \
        + (1.0 - KL_INTERP) * np.mean(ce) + nce_mean \
        + np.mean(foc) + np.mean(adv)
    return np.asarray([loss], dtype=np.float32)


def kernel(**inputs):
    in_maps = _prep_inputs(**inputs)
    out = run_bass_kernel_spmd(_get_nc(), in_maps,
                               core_ids=list(range(NCORES)))
    return _combine(out.results)


if __name__ == "__main__":
    rng = np.random.default_rng(0)
    ins = {
        "output": rng.standard_normal((B, C), dtype=np.float32),
        "target": rng.integers(0, C, size=(B,)),
        "master_net_pred": rng.standard_normal((B, C), dtype=np.float32),
        "feat_pooled": rng.standard_normal((B, D), dtype=np.float32),
        "feat_pooled_masked": rng.standard_normal((B, D), dtype=np.float32),
        "output_adv": rng.standard_normal((B, C), dtype=np.float32),
        "target_adv": rng.integers(0, C, size=(B,)),
    }
    print(kernel(**ins))


# revision 41
# speedup vs baseline: 1.0098x; 1.0098x over previous
"""Trainium2 Bass kernel for the combined loss (KL + CE + InfoNCE + focal + adv CE).

v3 strategy (8 NeuronCores, data-parallel over the batch):
  - InfoNCE exploits Gram symmetry: block-rows 0..3 compute circulant
    distances 0..32, block-rows 32..35 only 0..31 (their d=32 values come
    from the mirror block's colsum); all remaining mirrored distances come
    from PE ones-matmul column sums of the exp tiles.
  - Gram matmuls run in fp8(e4m3) DoubleRow mode: K=256 contraction in one
    pass at 2 cols/cycle.  Features are normalized, pre-scaled by
    sqrt(log2e/(32*T)) and packed [128, 2, EXT] on the host, so the PSUM
    values are y with exp(G/T) = 2^(32y).
  - Gram exp runs on ACT (scale=1/(s2*T)) with row-sum accumulators.
  - CE/KL/adv exp row-sums are split between ACT and a custom 2-instruction
    DVE exp pipeline (quartic poly in 2^y then 5 squarings, scale folded
    into the coefficients), balancing the two engines.
  - o/m/a ship as bf16; targets are gathered on the host (GO/GA stay host-side).
  - Colsum PSUM rows copy out through DVE; per-sample epilogue on host.
  - Positive-pair logits are read from the SBUF exp tiles (log on host) so no
    late DVE read ever pins a PSUM gram buffer; CE/KL units and colsum
    regions are spread across chunks to keep all engines fed.
"""

import numpy as np
import ml_dtypes
from operator import add as _add

import concourse.bacc as bacc
import concourse.tile as tile
from concourse import mybir
from concourse.bass_utils import run_bass_kernel_spmd

import concourse.dve_ops as DO
from concourse.dve_spec import (Spec, Src0, C0 as _C0, C1 as _C1, C2 as _C2,
                                C3 as _C3, Zero, One, lower as _dve_lower,
                                sq as _sq, _spill_c3_to_src1, _has_src1)
from concourse.dve_uop import DveOpSpec

F32 = mybir.dt.float32
BF16 = mybir.dt.bfloat16
FP8 = mybir.dt.float8e4
AF = mybir.ActivationFunctionType
ALU = mybir.AluOpType
DR = mybir.MatmulPerfMode.DoubleRow

NCORES = 8
B, C, D = 4096, 1000, 256
RB = B // NCORES          # 512 rows of the [B, C] tensors per core
NT = RB // 128            # 4 row-tiles per core
N2 = 2 * B                # 8192 infoNCE rows
NDIST = 33                # circulant distances d = 0..32 per block-row
SPAN = NDIST * 128        # 4224 columns per block-row sweep
L_ROWS = [0, 1, 2, 3, 32, 33, 34, 35]   # local block-row indices (all cores)
# d=32 blocks are computed only for l<4; the mirror rows (l>=32) get them
# via the colsum path, so those sweeps stop at distance 31.
EXT = 35 * 128 + 4096     # 8576 extended local columns
CHW = 1536                # gram/exp chunk width (3 PSUM banks)
NCHK = (EXT + CHW - 1) // CHW           # 6 chunks
RGW = 512                 # colsum accumulation region width (1 PSUM bank)
NREG = (EXT + RGW - 1) // RGW           # 17 regions (last one 384 wide)

KL_TEMP = 4.0
KL_INTERP = 0.5
NCE_TEMP = 0.07
LOG2E = float(np.log2(np.e))
S2 = LOG2E / (32.0 * NCE_TEMP)      # feature pre-scale^2; y = S2 * cos_sim
ACT_SCALE = 1.0 / (S2 * NCE_TEMP)   # exp(ACT_SCALE * y) = exp(G/T)
NEG_DIAG = -1.875                   # diag mask add (y_diag ~ -1.231)

# exp2 quartic: 2^y ~ c0*(1 + y(b1 + y(b2 + y(b3 + y*b4)))) on [-1.30, 0.67]
PB1, PB2, PB3, PB4 = 0.69336677, 0.24124203, 0.05543758, 0.00758271
PC0_32 = 0.9979927195289331         # c0^32

OMA_W = 6016              # o|m|a bf16 (2000 each) + 16 pad


def _register_dve(name, spec):
    if name in DO._SUB_OPCODE_FOR_NAME:
        return next(o for o in DO.OPS if o.name == name)
    op = DO.DveOp(name, spec, subdim=False, uops_sha={})
    DO.OPS.append(op)
    DO._SUB_OPCODE_FOR_NAME[name] = DO._CUSTOM_DVE_ROW_BASE + len(DO.OPS) - 1
    DO.CUSTOM_DVE_SPECS[name] = spec
    for ver in ("v3", "v4"):
        s = DveOpSpec(name=name, opcode=DO.get_dve_sub_opcode(name),
                      uops=_dve_lower(spec, ver=ver), rd1_en=_has_src1(spec))
        op.uops_sha[ver] = s.sha(ver)
    return op


def _ref_exp2pa(in0, in1, s0, s1, imm2):
    x = in0.astype(np.float32)
    return (1.0 + x * (s0 + x * (s1 + x * (imm2 + x * in1)))).astype(np.float32)


def _ref_exp2pb(in0, in1, s0, s1, imm2):
    b = ((in0.astype(np.float64) ** 32) * s0).astype(np.float32)
    return b, b.reshape(b.shape[0], -1).sum(axis=-1, keepdims=True).astype(
        np.float32)


EXP2PA = _register_dve("EXP2PA_ANT", Spec(
    body=_spill_c3_to_src1(
        One + Src0 * (_C0 + Src0 * (_C1 + Src0 * (_C2 + Src0 * _C3)))),
    reference=_ref_exp2pa))
EXP2PB = _register_dve("EXP2PB_ANT", Spec(
    body=_sq(_sq(_sq(_sq(_sq(Src0))))) * _C0, accum=_add, accum_init=Zero,
    reference=_ref_exp2pb))


def _poly_coefs(scale):
    """Fold an input pre-scale into the exp2 quartic: R(s*x) coefficients."""
    s = float(scale)
    return PB1 * s, PB2 * s * s, PB3 * s ** 3, PB4 * s ** 4


def _pair_table():
    """(l, c, a, b) for every (block-row, chunk) intersection, chunk-major."""
    pairs = []
    for c in range(NCHK):
        c0, c1 = CHW * c, min(CHW * (c + 1), EXT)
        for l in L_ROWS:
            s = 128 * l
            e = 128 * l + (SPAN if l < 4 else 4096)
            a, b = max(c0, s), min(c1, e)
            if b > a:
                pairs.append((l, c, a, b))
    return pairs


PAIRS = _pair_table()
NSLOT = len(PAIRS)        # 28


def _region_pieces():
    """region -> list of (pair_idx, p0, p1) colsum pieces; a full-covering
    piece (if any) is moved to the front so no zero-init matmul is needed."""
    reg = {}
    for idx, (l, c, a, b) in enumerate(PAIRS):
        a2 = max(a, 128 * l + 128)        # exclude d=0 (diagonal block)
        # l<4: include d=32 in the colsum (it feeds the l+32 mirror rows);
        # l>=32: sweep already stops at d=31.
        b2 = min(b, 128 * l + 4096 + (128 if l < 4 else 0))
        if b2 <= a2:
            continue
        r0, r1 = a2 // RGW, (b2 - 1) // RGW
        for r in range(r0, r1 + 1):
            p0, p1 = max(a2, RGW * r), min(b2, RGW * (r + 1))
            reg.setdefault(r, []).append((idx, p0, p1))
    for r, pieces in reg.items():
        end = min(RGW * (r + 1), EXT)
        full = [k for k, (_, p0, p1) in enumerate(pieces)
                if p0 == RGW * r and p1 == end]
        if full:
            k = full[0]
            pieces.insert(0, pieces.pop(k))
    return reg


REGION_PIECES = _region_pieces()

# CE/KL exp units: (tile, kind); kind 0=exp(o), 1=exp(o/4), 2=exp(m/4)->em,
# 3=exp(a).  Stat slot column = 33 + kind*4 + tile (S1|ST|SM|SA), PP at 49+t.
CEKL_ON_ACT = {(0, 0), (0, 2), (1, 2), (2, 2), (3, 2)}   # 5 units on ACT
OUT_W = 53


def _build_module():
    nc = bacc.Bacc("TRN2", target_bir_lowering=False, debug=False)

    oma_d = nc.dram_tensor("oma", [RB, OMA_W], mybir.dt.uint8,
                           kind="ExternalInput")
    hp_d = nc.dram_tensor("hp", [128, 2 * EXT], mybir.dt.uint8,
                          kind="ExternalInput")
    out_d = nc.dram_tensor("out", [128, OUT_W], F32, kind="ExternalOutput")
    cs_d = nc.dram_tensor("cs", [1, EXT], F32, kind="ExternalOutput")

    # packed constants: ident f32 | identb bf16 | negidb bf16 | onesb bf16 |
    # zerob bf16 | poly consts f32 (b4 variants)
    cpack = np.concatenate([
        np.eye(128, dtype=np.float32).view(np.uint8).reshape(128, -1),
        np.eye(128).astype(ml_dtypes.bfloat16).view(np.uint8).reshape(128, -1),
        (NEG_DIAG * np.eye(128)).astype(ml_dtypes.bfloat16).view(
            np.uint8).reshape(128, -1),
        np.ones((128, 128)).astype(ml_dtypes.bfloat16).view(
            np.uint8).reshape(128, -1),
        np.zeros((128, RGW)).astype(ml_dtypes.bfloat16).view(
            np.uint8).reshape(128, -1),
        np.tile(np.array([[_poly_coefs(1.0)[3],
                           _poly_coefs(LOG2E / 32.0)[3],
                           _poly_coefs(LOG2E / 128.0)[3]]], np.float32),
                (128, 1)).view(np.uint8).reshape(128, -1),
    ], axis=1)
    cpack_d = nc.inline_tensor(cpack, "cpack_c")

    with tile.TileContext(nc) as tc:
        with (
            tc.tile_pool(name="persist", bufs=1) as persist,
            tc.tile_pool(name="io", bufs=1) as iop,
            tc.tile_pool(name="em", bufs=4) as emp,
            tc.tile_pool(name="scr", bufs=2) as scrp,
            tc.tile_pool(name="qt", bufs=2) as qtp,
            tc.tile_pool(name="et", bufs=16) as etp,
            tc.tile_pool(name="vec", bufs=1) as vecp,
            tc.tile_pool(name="gp", bufs=2, space="PSUM") as gpp,
            tc.tile_pool(name="cs", bufs=2, space="PSUM") as csp,
        ):
            dma = nc.sync.dma_start

            cpack_t = persist.tile([128, cpack.shape[1]], mybir.dt.uint8,
                                   tag="cpack")
            ident_t = cpack_t[:, 0:512].bitcast(F32)
            identb_t = cpack_t[:, 512:768].bitcast(BF16)
            negidb_t = cpack_t[:, 768:1024].bitcast(BF16)
            onesb_t = cpack_t[:, 1024:1280].bitcast(BF16)
            zerob_t = cpack_t[:, 1280:1280 + 2 * RGW].bitcast(BF16)
            b4c_t = cpack_t[:, 1280 + 2 * RGW:1280 + 2 * RGW + 12].bitcast(F32)

            hp8 = persist.tile([128, 2, EXT], FP8, tag="hp8")
            hp8u = hp8.bitcast(mybir.dt.uint8)

            oma_ts, o_ts, m_ts, a_ts = [], [], [], []
            for t in range(NT):
                oma_t = iop.tile([128, OMA_W], mybir.dt.uint8, tag=f"oma{t}")
                oma_ts.append(oma_t)
                o_ts.append(oma_t[:, 0:2000].bitcast(BF16))
                m_ts.append(oma_t[:, 2000:4000].bitcast(BF16))
                a_ts.append(oma_t[:, 4000:6000].bitcast(BF16))

            # progressive feature pieces so the first gram starts early
            P0, P1, P2 = 512, 1536, 4608
            dma(out=cpack_t[:], in_=cpack_d[:])

            def dma_hp(lo, hi):
                for j in (0, 1):
                    dma(out=hp8u[:, j:j + 1, lo:hi],
                        in_=hp_d[:, j * EXT + lo:j * EXT + hi])

            def dma_oma(t, lo, hi):
                # split across queues so one tile isn't serialized on a ring
                rsl = slice(t * 128, (t + 1) * 128)
                step = (hi - lo + 1) // 2
                for x in range(lo, hi, step):
                    e = min(x + step, hi)
                    dma(out=oma_ts[t][:, x:e], in_=oma_d[rsl, x:e])

            dma_oma(0, 0, 2000)     # o of tile 0: first ACT work
            dma_hp(0, P0)
            dma_oma(0, 2000, 4000)
            dma_hp(P0, P1)
            dma_oma(0, 4000, 6016)
            dma_oma(1, 0, 4000)
            dma_hp(P1, P2)
            dma_oma(1, 4000, 6016)
            dma_oma(2, 0, 4000)
            dma_oma(3, 0, 4000)
            dma_hp(P2, EXT)
            dma_oma(2, 4000, 6016)
            dma_oma(3, 4000, 6016)

            out_sb = vecp.tile([128, OUT_W], F32, tag="out_sb")
            rs_sl = out_sb[:, 0:NSLOT]
            rs_x = out_sb[:, 28:29]          # second slot of the split pair 0
            pos_sb = out_sb[:, 29:33]
            st_sb = out_sb[:, 33:OUT_W]
            cs_sb = vecp.tile([1, EXT], F32, tag="cs_sb")
            pscr = vecp.tile([128, 128], F32, tag="pscr")
            dummy_a = vecp.tile([128, 1000], BF16, tag="dummy_a")  # ACT-only
            dummy_v = vecp.tile([128, 1000], BF16, tag="dummy_v")  # DVE-only

            et_tiles = {}
            em_ts = {}

            def emit_gram(idx):
                l, c, a, b = PAIRS[idx]
                w = b - a
                s_l = 128 * l
                gp = gpp.tile([128, CHW], F32, tag="gp")
                et_tiles[idx] = (gp, None)
                lhsT = hp8[:, :, s_l:s_l + 128]
                for sub in range(0, w, 512):
                    n = min(512, w - sub)
                    d0 = a + sub <= s_l < a + sub + n
                    nc.tensor.matmul(gp[:, sub:sub + n], lhsT,
                                     hp8[:, :, a + sub:a + sub + n],
                                     perf_mode=DR, start=True, stop=not d0,
                                     skip_group_check=True)
                    if d0:
                        off = s_l - a
                        nc.tensor.matmul(gp[:, off:off + 128], negidb_t[:],
                                         identb_t[:], start=False, stop=True,
                                         skip_group_check=True)
                return gp

            def emit_pos(idx):
                # diag of the d=32 block, read from the SBUF exp tile (NOT
                # the PSUM gram — a late DVE read there pins a PSUM buf and
                # stalls the PE/ACT pipeline); host takes log() to recover
                # the positive-pair logit.
                l, c, a, b = PAIRS[idx]
                p0 = 128 * l + 4096
                if l < 4 and a <= p0 < b:
                    off = p0 - a
                    e_t = et_tiles[idx][1]
                    nc.vector.scalar_tensor_tensor(
                        out=pscr[:], in0=e_t[:, off:off + 128], scalar=1.0,
                        in1=ident_t[:], op0=ALU.mult, op1=ALU.mult,
                        accum_out=pos_sb[:, l:l + 1])

            def emit_exp_act(idx, gp, split=False):
                l, c, a, b = PAIRS[idx]
                w = b - a
                e_t = etp.tile([128, CHW], BF16, tag="et")
                et_tiles[idx] = (gp, e_t)
                if split:
                    nc.scalar.activation(e_t[:, 0:512], gp[:, 0:512], AF.Exp,
                                         scale=ACT_SCALE, accum_out=rs_x[:])
                    nc.scalar.activation(e_t[:, 512:w], gp[:, 512:w], AF.Exp,
                                         scale=ACT_SCALE,
                                         accum_out=rs_sl[:, idx:idx + 1])
                else:
                    nc.scalar.activation(e_t[:, :w], gp[:, :w], AF.Exp,
                                         scale=ACT_SCALE,
                                         accum_out=rs_sl[:, idx:idx + 1])

            def emit_cekl_unit(t, kind):
                src = (o_ts[t], o_ts[t], m_ts[t], a_ts[t])[kind]
                slot = st_sb[:, kind * 4 + t:kind * 4 + t + 1]
                on_act = (t, kind) in CEKL_ON_ACT
                if kind == 2:
                    em_t = emp.tile([128, 1000], BF16, tag="em")
                    em_ts[t] = em_t
                    dst = em_t[:]
                else:
                    dst = (dummy_a if on_act else dummy_v)[:, 0:1000]
                if on_act:
                    scale = 1.0 if kind in (0, 3) else 0.25
                    nc.scalar.activation(dst, src[:], AF.Exp, scale=scale,
                                         accum_out=slot)
                else:
                    s = LOG2E / 32.0 if kind in (0, 3) else LOG2E / 128.0
                    b1, b2, b3, _ = _poly_coefs(s)
                    b4col = 1 if kind in (0, 3) else 2
                    q_t = qtp.tile([128, 1000], F32, tag="q")
                    nc.vector._custom_dve(
                        EXP2PA, out=q_t[:], in0=src[:],
                        in1=b4c_t[:, b4col:b4col + 1], s0=b1, s1=b2, imm2=b3)
                    nc.vector._custom_dve(
                        EXP2PB, out=dst, in0=q_t[:], s0=PC0_32,
                        accum_out=slot)

            def emit_cekl_pp(t):
                d_t = scrp.tile([128, 1000], BF16, tag="d")
                nc.vector.tensor_sub(d_t[:], m_ts[t][:], o_ts[t][:])
                nc.vector.scalar_tensor_tensor(
                    out=dummy_v[:, 0:1000], in0=d_t[:], scalar=1.0,
                    in1=em_ts[t][:], op0=ALU.mult, op1=ALU.mult,
                    accum_out=st_sb[:, 16 + t:17 + t])

            def emit_colsum_region(r):
                if r not in REGION_PIECES:
                    return
                pieces = REGION_PIECES[r]
                end = min(RGW * (r + 1), EXT)
                w = end - RGW * r
                ct = csp.tile([128, RGW], F32, tag="cs")
                full0 = (pieces[0][1] == RGW * r and pieces[0][2] == end)
                if not full0:
                    nc.tensor.matmul(ct[:, 0:w], onesb_t[:], zerob_t[:, 0:w],
                                     start=True, stop=False,
                                     skip_group_check=True)
                for k, (idx, p0_, p1_) in enumerate(pieces):
                    _, _, a, _ = PAIRS[idx]
                    e_t = et_tiles[idx][1]
                    nc.tensor.matmul(
                        ct[:, p0_ - RGW * r:p1_ - RGW * r],
                        onesb_t[:], e_t[:, p0_ - a:p1_ - a],
                        start=(k == 0 and full0),
                        stop=(k == len(pieces) - 1),
                        skip_group_check=True)
                nc.vector.tensor_copy(
                    cs_sb[0:1, RGW * r:end], ct[0:1, 0:w])
                dma(out=cs_d[0:1, RGW * r:end],
                    in_=cs_sb[0:1, RGW * r:end])

            # unit emission order: DVE units early (DMA-dependent only) in
            # tile order matching DMA arrival; ACT units fill gram gaps.
            dve_units = [(0, 1), (0, 3), (1, 1), (1, 0), (1, 3), (2, 1),
                         (2, 0), (2, 3), (3, 1), (3, 0), (3, 3)]
            act_units = [(1, 2), (2, 2), (3, 2)]
            pair_of_chunk = [[i for i, p in enumerate(PAIRS) if p[1] == c]
                             for c in range(NCHK)]
            pp_done = 0
            for c in range(NCHK):
                for k, idx in enumerate(pair_of_chunk[c]):
                    gp = emit_gram(idx)
                    # spread the previous chunk's colsum regions across this
                    # chunk's pairs so PE work stays smooth
                    if c >= 1 and k < 3:
                        emit_colsum_region(3 * (c - 1) + k)
                    emit_exp_act(idx, gp, split=(idx == 0))
                    emit_pos(idx)
                    if idx == 0:
                        # gram exp first (earliest data), then tile-0 cekl
                        emit_cekl_unit(0, 0)
                        emit_cekl_unit(0, 2)
                        for _ in range(2):
                            if dve_units:
                                emit_cekl_unit(*dve_units.pop(0))
                    elif k == len(pair_of_chunk[c]) // 2 and c >= 1:
                        if c >= 2 and act_units:
                            emit_cekl_unit(*act_units.pop(0))
                        for _ in range(2):
                            if dve_units:
                                emit_cekl_unit(*dve_units.pop(0))
                if c >= 1:
                    if dve_units:
                        emit_cekl_unit(*dve_units.pop(0))
                    if c >= 2 and pp_done < NT and pp_done in em_ts:
                        emit_cekl_pp(pp_done)
                        pp_done += 1
            for r in range(3 * (NCHK - 1), NREG):
                emit_colsum_region(r)
            for u in act_units:
                emit_cekl_unit(*u)
            for u in dve_units:
                emit_cekl_unit(*u)
            while pp_done < NT:
                if pp_done in em_ts:
                    emit_cekl_pp(pp_done)
                pp_done += 1

            dma(out=out_d[:], in_=out_sb[:])

    nc.compile()
    return nc


_NC = None


def _get_nc():
    global _NC
    if _NC is None:
        _NC = _build_module()
    return _NC


_HOST = {}


def _prep_inputs(output, target, master_net_pred, feat_pooled,
                 feat_pooled_masked, output_adv, target_adv):
    o = np.asarray(output, dtype=np.float32)
    m = np.asarray(master_net_pred, dtype=np.float32)
    a = np.asarray(output_adv, dtype=np.float32)
    tg = np.asarray(target).astype(np.int64)
    ta = np.asarray(target_adv).astype(np.int64)
    f0 = np.asarray(feat_pooled, dtype=np.float32)
    f1 = np.asarray(feat_pooled_masked, dtype=np.float32)
    feats = np.concatenate([f0, f1], axis=0)  # [2B, D]
    feats = feats / np.linalg.norm(feats, axis=1, keepdims=True)
    feats = feats * np.float32(np.sqrt(S2))

    _HOST["GO"] = np.take_along_axis(o, tg[:, None], axis=1)[:, 0]
    _HOST["GA"] = np.take_along_axis(a, ta[:, None], axis=1)[:, 0]

    o_bf = o.astype(ml_dtypes.bfloat16)
    m_bf = m.astype(ml_dtypes.bfloat16)
    a_bf = a.astype(ml_dtypes.bfloat16)

    in_maps = []
    for cc in range(NCORES):
        sl = slice(cc * RB, (cc + 1) * RB)
        rolled = np.roll(feats, -RB * cc, axis=0)
        ext = np.concatenate([rolled, rolled[:EXT - N2]], axis=0)  # [8704, D]
        f8 = np.ascontiguousarray(ext.T).astype(ml_dtypes.float8_e4m3)
        hp = np.concatenate([f8[0:128], f8[128:256]], axis=1)  # [128, 2*EXT]
        oma = np.zeros((RB, OMA_W), dtype=np.uint8)
        oma[:, 0:2000] = np.ascontiguousarray(o_bf[sl]).view(np.uint8)
        oma[:, 2000:4000] = np.ascontiguousarray(m_bf[sl]).view(np.uint8)
        oma[:, 4000:6000] = np.ascontiguousarray(a_bf[sl]).view(np.uint8)
        in_maps.append({"oma": oma, "hp": hp.view(np.uint8)})
    return in_maps


def _combine(results):
    S = np.zeros(N2, dtype=np.float64)
    pos_full = np.zeros(N2, dtype=np.float64)
    arp = np.arange(128)
    for cc, rr in enumerate(results):
        rs = rr["out"][:, 0:NSLOT].astype(np.float64)
        rs[:, 0] += rr["out"][:, 28].astype(np.float64)
        cs = rr["cs"].reshape(-1).astype(np.float64)   # [EXT]
        pos = rr["out"][:, 29:33].astype(np.float64)
        for idx, (l, c, a, b) in enumerate(PAIRS):
            rows = (RB * cc + 128 * l + arp) % N2
            np.add.at(S, rows, rs[:, idx])
        gcols = (np.arange(EXT) + RB * cc) % N2
        np.add.at(S, gcols, cs)
        for l in range(4):
            i = RB * cc + 128 * l + arp
            pos_full[i] = pos[:, l]
            pos_full[i + B] = pos[:, l]
    pos_logit = np.log(pos_full)   # pos slots hold exp(G/T) from the exp tile
    nce_mean = float(np.mean(np.log(S) - pos_logit))

    # CE / KL / focal / adv from per-row stats
    sts = [r["out"][:, 33:OUT_W] for r in results]
    S1 = np.concatenate([st[:, 0:4].T.reshape(-1) for st in sts])
    ST = np.concatenate([st[:, 4:8].T.reshape(-1) for st in sts])
    SM = np.concatenate([st[:, 8:12].T.reshape(-1) for st in sts])
    SA = np.concatenate([st[:, 12:16].T.reshape(-1) for st in sts])
    PP = np.concatenate([st[:, 16:20].T.reshape(-1) for st in sts])
    S1, ST, SM, SA, PP = (x.astype(np.float64)
                          for x in (S1, ST, SM, SA, PP))
    GO = _HOST["GO"].astype(np.float64)
    GA = _HOST["GA"].astype(np.float64)
    ce = np.log(S1) - GO
    adv = np.log(SA) - GA
    kl = PP / (KL_TEMP * SM) - np.log(SM) + np.log(ST)
    pt = np.exp(-ce)
    gamma = np.where(pt < 0.2, 5.0, np.where(pt < 0.5, 3.0, 1.0))
    foc = ((1.0 - pt) ** gamma) * ce
    loss = (KL_INTERP * KL_TEMP * KL_TEMP) * np.mean(kl) / C \
        + (1.0 - KL_INTERP) * np.mean(ce) + nce_mean \
        + np.mean(foc) + np.mean(adv)
    return np.asarray([loss], dtype=np.float32)


def kernel(**inputs):
    in_maps = _prep_inputs(**inputs)
    out = run_bass_kernel_spmd(_get_nc(), in_maps,
                               core_ids=list(range(NCORES)))
    return _combine(out.results)


if __name__ == "__main__":
    rng = np.random.default_rng(0)
    ins = {
        "output": rng.standard_normal((B, C), dtype=np.float32),
        "target": rng.integers(0, C, size=(B,)),
        "master_net_pred": rng.standard_normal((B, C), dtype=np.float32),
        "feat_pooled": rng.standard_normal((B, D), dtype=np.float32),
        "feat_pooled_masked": rng.standard_normal((B, D), dtype=np.float32),
        "output_adv": rng.standard_normal((B, C), dtype=np.float32),
        "target_adv": rng.integers(0, C, size=(B,)),
    }
    print(kernel(**ins))
